# revision 6
# baseline (speedup 1.0000x reference)
"""Chamfer loss kernel for Trainium2 (8 NeuronCores, SPMD).

Problem: B=4, N=M=8192, D=64 (fp32 in / fp32 scalar out).
  dist[b,n,m] = ||f[b,n] - f_[b,m]||^2
  out = mean_b( mean_n min_m dist + mean_m min_n dist )

Sharding: core c handles batch c//2, row-half c%2 (4096 rows x 8192 cols
of the distance matrix).

Device dataflow per core (E/D hybrid, tiles-outer, 2048-wide groups):
  - matmul (fp16, K=66): lhsT = [-2*f^T ; p ; 1], rhs = [f_^T ; 1 ; q-SHIFT]
    so PSUM tile = dist - SHIFT directly.
  - E-tiles (~27/32): ScalarE drains PSUM via Exp((S-d)/T) -> bf16 tile; the
    ACT accumulator emits row-sums as a side effect (row pass rides the
    drain).  DVE accumulates the column-MAX of the exp tiles at 2x rate
    (bf16): max(exp) = exp(-min), so the host recovers exact col-mins via
    S - T*log(max).  Row mins come from S - T*log(sum) with a small
    (~-0.2 abs) softmin bias -- negligible at rel-tol 2e-2.
  - D-tiles (~5/32): DVE consumes PSUM directly (1x): tensor_tensor min
    into the fp16 col accumulator C, and tensor_reduce min for row mins.
    No PSUM->SBUF cast exists for these tiles; they exist to offload the
    ScalarE, which otherwise would drain everything.
  Engine balance: ACT ~ exp of E share, DVE ~ E col-max @2x + D 2x1x psum
  passes, PE ~ 512 base matmuls (no extra work).
"""

import os

import numpy as np

import concourse.bass as bass
import concourse.mybir as mybir
import concourse.tile as tile
from concourse import bacc
from concourse.bass import ts
from concourse.bass_utils import run_bass_kernel_spmd

B, N, M, D = 4, 8192, 8192, 64
N_CORES = 8
ROWS = N // 2          # rows per core (half a batch)
SHIFT = 48.0           # fp16 centering for the D-path
S_LSE = 45.0           # LSE shift (absolute dist units)
T_LSE = 1.0            # LSE temperature

P = 128                # n-tile height
GW = 2048              # group width (psum tile = 4 banks)
MB = 512               # matmul free width (1 psum bank)
N_TILES = ROWS // P    # 32
N_G = M // GW          # 4 groups

D_COUNT = int(os.environ.get("CHAMFER_D_COUNT", "5"))

LAST_RESULTS = None    # test.py reads exec_time_ns / profile from here


def _d_tiles(n_tiles, d_count):
    if d_count <= 0:
        return set()
    return {int(round(k * n_tiles / d_count)) % n_tiles for k in range(d_count)}


def _build_program():
    K = D + 2
    f16 = mybir.dt.float16
    bf16 = mybir.dt.bfloat16
    f32 = mybir.dt.float32
    mmin = mybir.AluOpType.min
    mmax = mybir.AluOpType.max

    d_set = _d_tiles(N_TILES, D_COUNT)

    nc = bacc.Bacc()
    lhs_d = nc.dram_tensor("lhs", [K, ROWS], f16, kind="ExternalInput")
    rhs_d = nc.dram_tensor("rhs", [K, M], f16, kind="ExternalInput")
    col_d = nc.dram_tensor("colmins", [P, M], f16, kind="ExternalOutput")
    colx_d = nc.dram_tensor("colmaxe", [P, M], bf16, kind="ExternalOutput")
    rm_d = nc.dram_tensor("rowmins", [P, N_TILES * N_G], f32, kind="ExternalOutput")
    rs_d = nc.dram_tensor("rowsums", [P, N_TILES * N_G], f32, kind="ExternalOutput")

    with tile.TileContext(nc) as tc:
        with (
            tc.tile_pool(name="const", bufs=1) as cpool,
            tc.tile_pool(name="ebuf", bufs=5) as epool,
            tc.tile_pool(name="psb", bufs=2, space="PSUM") as pspool,
        ):
            lhs_sb = cpool.tile([K, ROWS], f16)
            rhs_sb = cpool.tile([K, M], f16)
            # chunked loads so the first matmuls start early
            for c in range(0, min(GW, ROWS), MB):
                nc.sync.dma_start(lhs_sb[:, c:c + MB], lhs_d[:, c:c + MB])
            for c in range(GW, ROWS, GW):
                e = min(c + GW, ROWS)
                nc.sync.dma_start(lhs_sb[:, c:e], lhs_d[:, c:e])
            for c in range(0, min(GW, M), MB):
                nc.sync.dma_start(rhs_sb[:, c:c + MB], rhs_d[:, c:c + MB])
            for c in range(GW, M, GW):
                e = min(c + GW, M)
                nc.sync.dma_start(rhs_sb[:, c:e], rhs_d[:, c:e])

            C = cpool.tile([P, M], f16)                  # D col-min accumulator
            CX = cpool.tile([P, M], bf16)                # E col-max(exp) accumulator
            RM = cpool.tile([P, N_TILES * N_G], f32)     # D row mins
            RS = cpool.tile([P, N_TILES * N_G], f32)     # E row sums
            biasT = cpool.tile([P, 1], f32)

            nc.vector.memset(C[:], 30000.0)
            nc.vector.memset(CX[:], 0.0)
            nc.gpsimd.memset(biasT[:], (S_LSE - SHIFT) / T_LSE)
            nc.gpsimd.memset(RM[:], 0.0)
            nc.gpsimd.memset(RS[:], 0.0)

            for i in range(N_TILES):
                lhs_i = lhs_sb[:, ts(i, P)]
                for g in range(N_G):
                    ps = pspool.tile([P, GW], f32)
                    for jj in range(GW // MB):
                        j = g * (GW // MB) + jj
                        nc.tensor.matmul(
                            ps[:, ts(jj, MB)],
                            lhs_i,
                            rhs_sb[:, ts(j, MB)],
                            start=True,
                            stop=True,
                        )
                    slot = i * N_G + g
                    if i in d_set:
                        nc.vector.tensor_tensor(
                            C[:, ts(g, GW)], ps[:], C[:, ts(g, GW)], mmin
                        )
                        nc.vector.tensor_reduce(
                            RM[:, slot:slot + 1], ps[:], mybir.AxisListType.X, mmin
                        )
                    else:
                        ebuf = epool.tile([P, GW], bf16)
                        nc.scalar.activation(
                            ebuf[:], ps[:], mybir.ActivationFunctionType.Exp,
                            bias=biasT[:], scale=-1.0 / T_LSE,
                            accum_out=RS[:, slot:slot + 1],
                        )
                        nc.vector.tensor_tensor(
                            CX[:, ts(g, GW)], ebuf[:], CX[:, ts(g, GW)], mmax
                        )

            for g in range(N_G):
                nc.sync.dma_start(col_d[:, ts(g, GW)], C[:, ts(g, GW)])
                nc.sync.dma_start(colx_d[:, ts(g, GW)], CX[:, ts(g, GW)])
            nc.sync.dma_start(rm_d[:, :], RM[:])
            nc.sync.dma_start(rs_d[:, :], RS[:])

    nc.finalize()
    return nc


_PROGRAM_CACHE = {}


def _get_program():
    key = (D_COUNT,)
    if key not in _PROGRAM_CACHE:
        _PROGRAM_CACHE[key] = _build_program()
    return _PROGRAM_CACHE[key]


def _prep_core_inputs(f, f_, core):
    """Host-side shard + layout: build augmented lhs/rhs for one core."""
    b, h = divmod(core, 2)
    fh = f[b, h * ROWS: (h + 1) * ROWS]          # [ROWS, D]
    g = f_[b]                                     # [M, D]
    p = np.einsum("nd,nd->n", fh, fh, dtype=np.float32)
    q = np.einsum("md,md->m", g, g, dtype=np.float32)

    K = D + 2
    lhs = np.empty((K, ROWS), np.float16)
    lhs[:D] = (-2.0 * fh.T).astype(np.float16)
    lhs[D] = p.astype(np.float16)
    lhs[D + 1] = 1.0

    rhs = np.empty((K, M), np.float16)
    rhs[:D] = g.T.astype(np.float16)
    rhs[D] = 1.0
    rhs[D + 1] = (q - SHIFT).astype(np.float16)
    return {"lhs": lhs, "rhs": rhs}


def _core_row_col_mins(res_core, d_set):
    """Recover per-core row mins [ROWS] and col mins [M] (absolute units)."""
    rm = res_core["rowmins"].reshape(P, N_TILES, N_G)
    rs = res_core["rowsums"].reshape(P, N_TILES, N_G)
    row = np.empty((N_TILES, P), np.float32)
    for i in range(N_TILES):
        if i in d_set:
            row[i] = rm[:, i, :].min(axis=1) + SHIFT
        else:
            s = rs[:, i, :].sum(axis=1)
            row[i] = S_LSE - T_LSE * np.log(np.maximum(s, 1e-38))
    rows = row.reshape(N_TILES * P)

    cx = res_core["colmaxe"].astype(np.float32).max(axis=0)
    col_e = S_LSE - T_LSE * np.log(np.maximum(cx, 1e-38))
    if d_set:
        col_dd = res_core["colmins"].astype(np.float32).min(axis=0) + SHIFT
        cols = np.minimum(col_dd, col_e)
    else:
        cols = col_e
    return rows, cols


def kernel(f, f_):
    global LAST_RESULTS
    f = np.asarray(f, dtype=np.float32)
    f_ = np.asarray(f_, dtype=np.float32)

    in_maps = [_prep_core_inputs(f, f_, c) for c in range(N_CORES)]
    nc = _get_program()
    res = run_bass_kernel_spmd(
        nc,
        in_maps,
        list(range(N_CORES)),
        trace=bool(int(os.environ.get("CHAMFER_TRACE", "0"))),
    )
    LAST_RESULTS = res

    d_set = _d_tiles(N_TILES, D_COUNT)
    total = 0.0
    for b in range(B):
        r0, c0 = _core_row_col_mins(res.results[2 * b], d_set)
        r1, c1 = _core_row_col_mins(res.results[2 * b + 1], d_set)
        rm = np.concatenate([r0, r1])
        cm = np.minimum(c0, c1)
        total += rm.mean() + cm.mean()
    return np.asarray(total / B, dtype=np.float32)


# revision 12
# speedup vs baseline: 1.0510x; 1.0510x over previous
"""Chamfer loss kernel for Trainium2 (8 NeuronCores, SPMD).

Problem: B=4, N=M=8192, D=64 (fp32 in / fp32 scalar out).
  dist[b,n,m] = ||f[b,n] - f_[b,m]||^2
  out = mean_b( mean_n min_m dist + mean_m min_n dist )

Sharding: core c handles batch c//2, row-half c%2 (4096 rows x 8192 cols
of the distance matrix).

Device dataflow per core (E/D hybrid, tiles-outer, 2048-wide groups):
  - matmul (fp16, K=66): lhsT = [-2*f^T ; p ; 1], rhs = [f_^T ; 1 ; q-SHIFT]
    so PSUM tile = dist - SHIFT directly.
  - E-tiles (~27/32): ScalarE drains PSUM via Exp((S-d)/T) -> bf16 tile; the
    ACT accumulator emits row-sums as a side effect (row pass rides the
    drain).  DVE accumulates the column-MAX of the exp tiles at 2x rate
    (bf16): max(exp) = exp(-min), so the host recovers exact col-mins via
    S - T*log(max).  Row mins come from S - T*log(sum) with a small
    (~-0.2 abs) softmin bias -- negligible at rel-tol 2e-2.
  - D-tiles (~5/32): DVE consumes PSUM directly (1x): tensor_tensor min
    into the fp16 col accumulator C, and tensor_reduce min for row mins.
    No PSUM->SBUF cast exists for these tiles; they exist to offload the
    ScalarE, which otherwise would drain everything.
  Engine balance: ACT ~ exp of E share, DVE ~ E col-max @2x + D 2x1x psum
  passes, PE ~ 512 base matmuls (no extra work).
"""

import os

import numpy as np

import concourse.bass as bass
import concourse.mybir as mybir
import concourse.tile as tile
from concourse import bacc
from concourse.bass import ts
from concourse.bass_utils import run_bass_kernel_spmd

B, N, M, D = 4, 8192, 8192, 64
N_CORES = 8
ROWS = N // 2          # rows per core (half a batch)
SHIFT = 48.0           # fp16 centering for the D-path
S_LSE = 45.0           # LSE shift (absolute dist units)
T_LSE = 1.0            # LSE temperature

P = 128                # n-tile height
GW = 2048              # group width (psum tile = 4 banks)
MB = 512               # matmul free width (1 psum bank)
N_TILES = ROWS // P    # 32
N_G = M // GW          # 4 groups

D_EVERY = int(os.environ.get("CHAMFER_D_EVERY", "6"))
D_SKIP_HEAD = 8        # no D cells in the first/last cells (startup/tail stalls)
D_SKIP_TAIL = 4

LAST_RESULTS = None    # test.py reads exec_time_ns / profile from here


def _d_cells():
    """Cell (tile*N_G+g) -> D-path?  Spread ~1/D_EVERY, avoiding head/tail."""
    n_cells = N_TILES * N_G
    if D_EVERY <= 0:
        return set()
    return {
        c for c in range(D_SKIP_HEAD, n_cells - D_SKIP_TAIL)
        if (c - D_SKIP_HEAD) % D_EVERY == 0
    }


def _build_program():
    K = D + 2
    f16 = mybir.dt.float16
    bf16 = mybir.dt.bfloat16
    f32 = mybir.dt.float32
    mmin = mybir.AluOpType.min
    mmax = mybir.AluOpType.max

    d_cells = _d_cells()

    nc = bacc.Bacc()
    lhs_d = nc.dram_tensor("lhs", [K, ROWS], f16, kind="ExternalInput")
    rhs_d = nc.dram_tensor("rhs", [K, M], f16, kind="ExternalInput")
    col_d = nc.dram_tensor("colmins", [P, M], f16, kind="ExternalOutput")
    colx_d = nc.dram_tensor("colmaxe", [P, M], bf16, kind="ExternalOutput")
    rm_d = nc.dram_tensor("rowmins", [P, N_TILES * N_G], f32, kind="ExternalOutput")
    rs_d = nc.dram_tensor("rowsums", [P, N_TILES * N_G], f32, kind="ExternalOutput")

    with tile.TileContext(nc) as tc:
        with (
            tc.tile_pool(name="const", bufs=1) as cpool,
            tc.tile_pool(name="ebuf", bufs=5) as epool,
            tc.tile_pool(name="psb", bufs=2, space="PSUM") as pspool,
        ):
            lhs_sb = cpool.tile([K, ROWS], f16)
            rhs_sb = cpool.tile([K, M], f16)
            # chunked loads so the first matmuls start early
            for c in range(0, min(GW, ROWS), MB):
                nc.sync.dma_start(lhs_sb[:, c:c + MB], lhs_d[:, c:c + MB])
            for c in range(GW, ROWS, GW):
                e = min(c + GW, ROWS)
                nc.sync.dma_start(lhs_sb[:, c:e], lhs_d[:, c:e])
            for c in range(0, min(GW, M), MB):
                nc.sync.dma_start(rhs_sb[:, c:c + MB], rhs_d[:, c:c + MB])
            for c in range(GW, M, GW):
                e = min(c + GW, M)
                nc.sync.dma_start(rhs_sb[:, c:e], rhs_d[:, c:e])

            C = cpool.tile([P, M], f16)                  # D col-min accumulator
            CX = cpool.tile([P, M], bf16)                # E col-max(exp) accumulator
            RM = cpool.tile([P, N_TILES * N_G], f32)     # D row mins
            RS = cpool.tile([P, N_TILES * N_G], f32)     # E row sums
            biasT = cpool.tile([P, 1], f32)

            nc.vector.memset(C[:], 30000.0)
            nc.vector.memset(CX[:], 0.0)
            nc.gpsimd.memset(biasT[:], (S_LSE - SHIFT) / T_LSE)
            nc.gpsimd.memset(RM[:], 0.0)
            nc.gpsimd.memset(RS[:], 0.0)

            for i in range(N_TILES):
                lhs_i = lhs_sb[:, ts(i, P)]
                for g in range(N_G):
                    ps = pspool.tile([P, GW], f32)
                    for jj in range(GW // MB):
                        j = g * (GW // MB) + jj
                        nc.tensor.matmul(
                            ps[:, ts(jj, MB)],
                            lhs_i,
                            rhs_sb[:, ts(j, MB)],
                            start=True,
                            stop=True,
                        )
                    slot = i * N_G + g
                    if slot in d_cells:
                        nc.vector.tensor_tensor(
                            C[:, ts(g, GW)], ps[:], C[:, ts(g, GW)], mmin
                        )
                        nc.vector.tensor_reduce(
                            RM[:, slot:slot + 1], ps[:], mybir.AxisListType.X, mmin
                        )
                    else:
                        ebuf = epool.tile([P, GW], bf16)
                        nc.scalar.activation(
                            ebuf[:], ps[:], mybir.ActivationFunctionType.Exp,
                            bias=biasT[:], scale=-1.0 / T_LSE,
                            accum_out=RS[:, slot:slot + 1],
                        )
                        nc.vector.tensor_tensor(
                            CX[:, ts(g, GW)], ebuf[:], CX[:, ts(g, GW)], mmax
                        )

            for g in range(N_G):
                nc.sync.dma_start(col_d[:, ts(g, GW)], C[:, ts(g, GW)])
                nc.sync.dma_start(colx_d[:, ts(g, GW)], CX[:, ts(g, GW)])
            nc.sync.dma_start(rm_d[:, :], RM[:])
            nc.sync.dma_start(rs_d[:, :], RS[:])

    nc.finalize()
    return nc


_PROGRAM_CACHE = {}


def _get_program():
    key = (D_EVERY,)
    if key not in _PROGRAM_CACHE:
        _PROGRAM_CACHE[key] = _build_program()
    return _PROGRAM_CACHE[key]


def _prep_core_inputs(f, f_, core):
    """Host-side shard + layout: build augmented lhs/rhs for one core."""
    b, h = divmod(core, 2)
    fh = f[b, h * ROWS: (h + 1) * ROWS]          # [ROWS, D]
    g = f_[b]                                     # [M, D]
    p = np.einsum("nd,nd->n", fh, fh, dtype=np.float32)
    q = np.einsum("md,md->m", g, g, dtype=np.float32)

    K = D + 2
    lhs = np.empty((K, ROWS), np.float16)
    lhs[:D] = (-2.0 * fh.T).astype(np.float16)
    lhs[D] = p.astype(np.float16)
    lhs[D + 1] = 1.0

    rhs = np.empty((K, M), np.float16)
    rhs[:D] = g.T.astype(np.float16)
    rhs[D] = 1.0
    rhs[D + 1] = (q - SHIFT).astype(np.float16)
    return {"lhs": lhs, "rhs": rhs}


def _core_row_col_mins(res_core, d_cells):
    """Recover per-core row mins [ROWS] and col mins [M] (absolute units)."""
    rm = res_core["rowmins"].reshape(P, N_TILES * N_G)
    rs = res_core["rowsums"].reshape(P, N_TILES * N_G)
    is_d = np.zeros(N_TILES * N_G, bool)
    for c in d_cells:
        is_d[c] = True
    # per-cell row-min estimates: D cells true mins, E cells LSE of the sums
    est = np.where(
        is_d[None, :],
        rm + SHIFT,
        S_LSE - T_LSE * np.log(np.maximum(rs, 1e-38)),
    )
    rows = est.reshape(P, N_TILES, N_G).min(axis=2).T.reshape(-1)

    cx = res_core["colmaxe"].astype(np.float32).max(axis=0)
    col_e = S_LSE - T_LSE * np.log(np.maximum(cx, 1e-38))
    col_dd = res_core["colmins"].astype(np.float32).min(axis=0) + SHIFT
    cols = np.minimum(col_dd, col_e)
    return rows, cols


def kernel(f, f_):
    global LAST_RESULTS
    f = np.asarray(f, dtype=np.float32)
    f_ = np.asarray(f_, dtype=np.float32)

    in_maps = [_prep_core_inputs(f, f_, c) for c in range(N_CORES)]
    nc = _get_program()
    res = run_bass_kernel_spmd(
        nc,
        in_maps,
        list(range(N_CORES)),
        trace=bool(int(os.environ.get("CHAMFER_TRACE", "0"))),
    )
    LAST_RESULTS = res

    d_cells = _d_cells()
    total = 0.0
    for b in range(B):
        r0, c0 = _core_row_col_mins(res.results[2 * b], d_cells)
        r1, c1 = _core_row_col_mins(res.results[2 * b + 1], d_cells)
        rm = np.concatenate([r0, r1])
        cm = np.minimum(c0, c1)
        total += rm.mean() + cm.mean()
    return np.asarray(total / B, dtype=np.float32)
